# revision 55
# baseline (speedup 1.0000x reference)
"""Tensor-parallel GQA attention prefill for 8 TRN2 NeuronCores.

Shards the 32 Q heads / 8 KV heads across 8 cores (4 Q heads + 1 KV head
per core, kv-groups intact). Each core computes its heads' attention and
a partial output projection; the host sums the 8 partials.

Device-side layout choices (all prepared host-side):
 - x is passed transposed (xT [dim, tok]) so the contraction dim of the
   QKV projections lands on SBUF partitions with contiguous DMA.
 - wq/wk rows are permuted within each head to [even dims, odd dims] so
   RoPE's interleaved pairs become two contiguous 64-partition blocks.
   The permutation cancels in q.k dot products.
 - Projections produce qT/kT [d, tok]; scores are computed transposed
   (scoresT [ktok, qtok]) so softmax sums use a ones-matmul and the AV
   matmul needs no transposes. v is produced via PE-transpose of vT.
 - Causality is exploited structurally: upper-triangle score tiles are
   never computed; diagonal tiles are masked with a host-provided 0/1
   mask multiplied after exp (exp is overflow-safe at this scale, so no
   max subtraction is needed).
 - All matmul operands are bf16 (same 1 cycle/row PE rate as float32r,
   but FWL weight loads, half the DMA traffic and half the SBUF);
   PSUM accumulation and the softmax denominator stay fp32.
 - q stripes stay resident in SBUF between the projection and attention
   phases (32KB/partition in bf16) instead of a DRAM round trip.
 - PSUM->SBUF out-tile evictions alternate ACT/DVE (Pool cannot touch
   PSUM); the softmax accumulation runs in bf16 on DVE.
 - LDWEIGHTS exposure (~35ns per stationary change, measured) is cut by
   sharing stationaries: attention runs two heads interleaved (score/AV
   matmuls for both heads reuse each kT/v chunk), and the out projection
   emits two adjacent column slices per attnT stationary, h-major.
 - Diagonal score/AV matmuls skip causally-dead 128-column blocks; the
   trimmed prob regions are zeroed by Pool-engine memsets.
"""

import math
from contextlib import ExitStack

import ml_dtypes
import numpy as np

import concourse.bass as bass
import concourse.mybir as mybir
import concourse.tile as tile
from concourse import bacc
from concourse.bass import ts, ds
from concourse.bass_utils import run_bass_kernel_spmd
from concourse.masks import make_identity

P = 128
DIM = 4096
T = 4096          # b*s tokens, b-major
B = 2
S = 2048
N_HEADS_LOCAL = 4     # q heads per core
HD = 128              # head dim
QD = N_HEADS_LOCAL * HD   # 512 local q dim
N_CORES = 8
STRIPE = 512          # token stripe for projections / q chunks
N_STRIPES = T // STRIPE       # 8
K_CHUNKS = DIM // P           # 32
TOK_CHUNKS = T // P           # 32
SCALE = 1.0 / math.sqrt(HD)

F32 = mybir.dt.float32
F32R = mybir.dt.float32r
BF16 = mybir.dt.bfloat16

_NC_CACHE = {}


def build_nc(loop_n: int = 1):
    nc = bacc.Bacc("TRN2", target_bir_lowering=False, debug=False)

    xT = nc.dram_tensor("xT", [DIM, T], BF16, kind="ExternalInput").ap()
    wqT = nc.dram_tensor("wqT", [DIM, QD], BF16, kind="ExternalInput").ap()
    wkT = nc.dram_tensor("wkT", [DIM, HD], BF16, kind="ExternalInput").ap()
    wvT = nc.dram_tensor("wvT", [DIM, HD], BF16, kind="ExternalInput").ap()
    woT = nc.dram_tensor("woT", [QD, DIM], BF16, kind="ExternalInput").ap()
    cosT = nc.dram_tensor("cosT", [64, T], F32, kind="ExternalInput").ap()
    sinT = nc.dram_tensor("sinT", [64, T], F32, kind="ExternalInput").ap()
    cmask = nc.dram_tensor("cmask", [P, 4, STRIPE], BF16, kind="ExternalInput").ap()
    out = nc.dram_tensor("out", [T, DIM], F32, kind="ExternalOutput").ap()

    with tile.TileContext(nc) as tc, ExitStack() as octx:
        # ---- tensors that live across phases ----
        resident = octx.enter_context(tc.tile_pool(name="resident", bufs=1))
        # per-stripe k/v tiles: an attention group's reads depend on exactly
        # the stripes it consumes, not the whole phase-1 sweep
        kT_st = [resident.tile([P, STRIPE], BF16, tag=f"kT{st}", name=f"kT{st}")
                 for st in range(N_STRIPES)]                      # 8KB/part
        v_st = [resident.tile([P, STRIPE // P, HD], BF16, tag=f"v{st}",
                              name=f"v{st}")
                for st in range(N_STRIPES)]                       # 8KB/part
        # q stripes stay in SBUF: [head][stripe] -> [128, 512] bf16
        q_sb = [[resident.tile([P, STRIPE], BF16, tag=f"q{h}_{st}",
                               name=f"q{h}_{st}")
                 for st in range(N_STRIPES)] for h in range(N_HEADS_LOCAL)]
        ones_sb = resident.tile([P, P], BF16, tag="ones")
        ones_f32 = resident.tile([P, P], F32, tag="ones_f32")
        ident_sb = resident.tile([P, P], F32, tag="ident")
        cmask_sb = resident.tile([P, 4, STRIPE], BF16, tag="cmask")
        nc.gpsimd.memset(ones_f32[:], 1.0)
        nc.vector.tensor_copy(ones_sb[:], ones_f32[:])
        make_identity(nc, ident_sb[:])
        nc.sync.dma_start(cmask_sb[:], cmask)

        # phase-2/3 working pools live in the outer scope: allocating them
        # inside the phase would stall on the phase-1 pool boundary (all of
        # phase 1's SBUF consumers must drain before the space is reusable)
        probs_pool = octx.enter_context(tc.tile_pool(name="probs", bufs=8))
        accpool = octx.enter_context(tc.tile_pool(name="acc", bufs=6))
        opool = octx.enter_context(tc.tile_pool(name="outt", bufs=8))
        # wo lives in the outer scope (bf16 leaves SBUF room) so the first
        # out-proj fillers right after the transition don't wait on a 4MB
        # DMA; the load itself is issued mid-phase-1, off the startup path
        wopool = octx.enter_context(tc.tile_pool(name="weights3", bufs=1))
        wo_sb = wopool.tile([P, N_HEADS_LOCAL, DIM], BF16, tag="wo")  # 32KB
        # rope inputs live in the outer scope so the last stripe's q1-3
        # ropes can be deferred into phase 2 (their tiles must survive the
        # phase-1 pool boundary)
        cspool = octx.enter_context(tc.tile_pool(name="cossin", bufs=1))
        evict = octx.enter_context(tc.tile_pool(name="evict", bufs=4))
        rtmp = octx.enter_context(tc.tile_pool(name="rope_tmp", bufs=2))

        def rope(dst_hi, dst_lo, src, cos_s, sin_s):
            # src [128, STRIPE] SBUF fp32: rows 0:64 = t0 (even dims),
            # 64:128 = t1. cos_s/sin_s are [128, STRIPE] with the same 64
            # rows duplicated into both halves so every tensor_tensor's
            # two SBUF inputs share a base partition (walrus NCC_IBIR297).
            t0, t1 = src[0:64, :], src[64:128, :]
            a = rtmp.tile([64, STRIPE], F32, tag="rt", name="ra")
            b_ = rtmp.tile([64, STRIPE], F32, tag="rt", name="rb")
            nc.vector.tensor_mul(a[:], t0, cos_s[0:64, :])
            nc.vector.tensor_mul(b_[:], t1, sin_s[64:128, :])
            nc.vector.tensor_sub(dst_hi, a[:], b_[:])
            c_ = rtmp.tile([64, STRIPE], F32, tag="rt", name="rc")
            d_ = rtmp.tile([64, STRIPE], F32, tag="rt", name="rd")
            nc.vector.tensor_mul(c_[:], t0, sin_s[0:64, :])
            nc.vector.tensor_mul(d_[:], t1, cos_s[64:128, :])
            nc.vector.tensor_add(dst_lo, c_[:], d_[:])

        deferred_ropes = []   # (h, st, qcop, cos_s, sin_s) emitted in phase 2

        if loop_n > 1:   # timing builds: repeat the whole body on-device
            octx.enter_context(tc.For_i(0, loop_n, 1))

        # ================= phase 1: projections + rope =================
        with ExitStack() as ctx:
            wpool = ctx.enter_context(tc.tile_pool(name="weights1", bufs=1))
            xpool = ctx.enter_context(tc.tile_pool(name="xk", bufs=3))
            qpsum = ctx.enter_context(tc.tile_pool(name="q_psum", bufs=4, space="PSUM"))
            kpsum = ctx.enter_context(tc.tile_pool(name="k_psum", bufs=1, space="PSUM"))
            vpsum = ctx.enter_context(tc.tile_pool(name="v_psum", bufs=2, space="PSUM"))
            tpsum = ctx.enter_context(tc.tile_pool(name="tr_psum", bufs=1, space="PSUM"))
            vt_pool = ctx.enter_context(tc.tile_pool(name="vt", bufs=2))

            wq_sb = wpool.tile([P, K_CHUNKS, QD], BF16, tag="wq")   # 32KB/part
            wk_sb = wpool.tile([P, K_CHUNKS, HD], BF16, tag="wk")   # 8KB
            wv_sb = wpool.tile([P, K_CHUNKS, HD], BF16, tag="wv")   # 8KB

            def _proj_chunk(st, k, xk, psq, psk, psv, prev_vt):
                if st == 0 and k % 2 == 0:
                    # paired per-chunk weight loads (2 chunks per transfer):
                    # matmul k waits only on its own pair, and half the
                    # per-DMA fixed cost on the startup-critical path
                    nc.sync.dma_start(
                        wq_sb[:, k:k + 2, :],
                        wqT[ds(k * P, 2 * P), :].rearrange("(j p) c -> p j c", p=P))
                    nc.sync.dma_start(
                        wk_sb[:, k:k + 2, :],
                        wkT[ds(k * P, 2 * P), :].rearrange("(j p) c -> p j c", p=P))
                    nc.sync.dma_start(
                        wv_sb[:, k:k + 2, :],
                        wvT[ds(k * P, 2 * P), :].rearrange("(j p) c -> p j c", p=P))
                st_first, st_last = (k == 0), (k == K_CHUNKS - 1)
                for h in range(N_HEADS_LOCAL):
                    nc.tensor.matmul(psq[h][:], wq_sb[:, k, ts(h, HD)], xk,
                                     start=st_first, stop=st_last)
                nc.tensor.matmul(psk[:], wk_sb[:, k, :], xk,
                                 start=st_first, stop=st_last)
                nc.tensor.matmul(psv[:], wv_sb[:, k, :], xk,
                                 start=st_first, stop=st_last)
                # previous stripe's v transposes: deps met long ago, sit
                # between accumulation matmuls without stalling PE
                if k == 0 and prev_vt is not None:
                    pvt, pvt_st = prev_vt
                    for j in range(STRIPE // P):
                        pstt = tpsum.tile([P, P], F32, tag="pst",
                                          name=f"pst{j}")
                        nc.tensor.transpose(pstt[:], pvt[:, ts(j, P)],
                                            ident_sb[:])
                        nc.scalar.copy(v_st[pvt_st][:, j, :], pstt[:])

            prev_vt = None
            for st in range(N_STRIPES):
                tok = ts(st, STRIPE)
                psq = [qpsum.tile([P, STRIPE], F32, tag="psq", name=f"psq{i}")
                       for i in range(N_HEADS_LOCAL)]
                psk = kpsum.tile([P, STRIPE], F32, tag="psk")
                psv = vpsum.tile([P, STRIPE], F32, tag="psv")
                for k2 in range(K_CHUNKS // 2):
                    # two k-chunks per DMA: halves the per-transfer fixed
                    # costs on the phase-1 feed path
                    xk2 = xpool.tile([P, 2, STRIPE], BF16, tag="xk")
                    nc.sync.dma_start(
                        xk2[:], xT[ds(k2 * 2 * P, 2 * P), tok].rearrange(
                            "(j p) t -> p j t", p=P))
                    for j in range(2):
                        k = 2 * k2 + j
                        _proj_chunk(st, k, xk2[:, j, :], psq, psk, psv,
                                    prev_vt)

                # evict PSUM -> SBUF fast on the Pool engine so next stripe's
                # matmuls get their PSUM banks back quickly
                kcop = evict.tile([P, STRIPE], F32, tag="kcop")
                nc.scalar.copy(kcop[:], psk[:])
                vt = vt_pool.tile([P, STRIPE], F32, tag="vt")
                nc.scalar.copy(vt[:], psv[:])
                qcop = []
                for h in range(N_HEADS_LOCAL):
                    qc_ = evict.tile([P, STRIPE], F32, tag="kcop",
                                     name=f"qcop{h}")
                    nc.scalar.copy(qc_[:], psq[h][:])
                    qcop.append(qc_)

                cos_s = cspool.tile([P, STRIPE], F32, tag="cos")
                sin_s = cspool.tile([P, STRIPE], F32, tag="sin")
                nc.sync.dma_start(cos_s[0:64, :], cosT[:, tok])
                nc.sync.dma_start(sin_s[0:64, :], sinT[:, tok])
                nc.scalar.copy(cos_s[64:128, :], cos_s[0:64, :])
                nc.scalar.copy(sin_s[64:128, :], sin_s[0:64, :])

                if st == 2:   # past the startup DMA crunch, long before use
                    for hh in range(N_HEADS_LOCAL):
                        nc.sync.dma_start(wo_sb[:, hh, :], woT[ts(hh, P), :])

                rope(kT_st[st][0:64, :], kT_st[st][64:128, :], kcop[:],
                     cos_s[:], sin_s[:])
                for h in range(N_HEADS_LOCAL):
                    if st == N_STRIPES - 1 and h >= 1:
                        # stripe 7's q1-3 feed only the very last attention
                        # blocks; deferring their ropes into phase 2 keeps
                        # ~18 TensorTensor ops out of the strict-FIFO DVE
                        # queue at the phase transition, so the first
                        # blocks' mask/acc (and thus AV matmuls) start
                        # immediately
                        deferred_ropes.append((h, st, qcop[h], cos_s, sin_s))
                    else:
                        rope(q_sb[h][st][0:64, :], q_sb[h][st][64:128, :],
                             qcop[h][:], cos_s[:], sin_s[:])
                prev_vt = (vt, st)

            # last stripe's v transposes
            pvt, pvt_st = prev_vt
            for j in range(STRIPE // P):
                pstt = tpsum.tile([P, P], F32, tag="pst", name=f"pstz{j}")
                nc.tensor.transpose(pstt[:], pvt[:, ts(j, P)], ident_sb[:])
                nc.scalar.copy(v_st[pvt_st][:, j, :], pstt[:])

        # ================= phase 2+3: attention + out proj =================
        with ExitStack() as ctx:
            atpool = ctx.enter_context(tc.tile_pool(name="attnT", bufs=1))
            spsum = ctx.enter_context(tc.tile_pool(name="s_psum", bufs=2, space="PSUM"))
            avpsum = ctx.enter_context(tc.tile_pool(name="av_psum", bufs=2, space="PSUM"))
            opsum = ctx.enter_context(tc.tile_pool(name="o_psum", bufs=2, space="PSUM"))
            cspsum = opsum  # colsum tiles share the out-proj psum slots

            # per-(b,qc) tiles so an out tile depends on exactly its 4 head
            # writes, not every norm traced before it
            attnT_bq = [[atpool.tile([P, N_HEADS_LOCAL, STRIPE], BF16,
                                     tag=f"attnT{b}_{qc}", name=f"attnT{b}_{qc}")
                         for qc in range(S // STRIPE)] for b in range(B)]  # 32KB

            DEPTH_PAIRS = 3   # score2 -> exp2 -> av pipeline depth (in kj pairs)
            from collections import deque
            ready_tiles = deque()

            def out_tile(tc32, np2):
                # two adjacent out-column slices per call, matmuls h-major:
                # consecutive matmuls share their attnT stationary operand,
                # halving the LDWEIGHTS traffic of the output projection
                b, qc = tc32 // (TOK_CHUNKS // 2), (tc32 % (TOK_CHUNKS // 2)) // 4
                ps_oA = opsum.tile([P, STRIPE], F32, tag="o", name="ps_oA")
                ps_oB = opsum.tile([P, STRIPE], F32, tag="o", name="ps_oB")
                for h in range(N_HEADS_LOCAL):
                    st_f, st_l = (h == 0), (h == N_HEADS_LOCAL - 1)
                    stat = attnT_bq[b][qc][:, h, ts(tc32 % 4, P)]
                    nc.tensor.matmul(ps_oA[:], stat,
                                     wo_sb[:, h, ts(2 * np2, STRIPE)],
                                     start=st_f, stop=st_l)
                    nc.tensor.matmul(ps_oB[:], stat,
                                     wo_sb[:, h, ts(2 * np2 + 1, STRIPE)],
                                     start=st_f, stop=st_l)
                o_sbA = opool.tile([P, STRIPE], F32, tag="o_sb", name="o_sbA")
                o_sbB = opool.tile([P, STRIPE], F32, tag="o_sb", name="o_sbB")
                # parallel PSUM evictions: ACT takes one, DVE the other
                nc.scalar.copy(o_sbA[:], ps_oA[:])
                nc.vector.tensor_copy(o_sbB[:], ps_oB[:])
                nc.sync.dma_start(out[ts(tc32, P), ts(2 * np2, STRIPE)],
                                  o_sbA[:])
                nc.sync.dma_start(out[ts(tc32, P), ts(2 * np2 + 1, STRIPE)],
                                  o_sbB[:])

            def filler(nmax):
                for _ in range(min(nmax, len(ready_tiles))):
                    out_tile(*ready_tiles.popleft())

            def attn_group2(b, h0, qc, early=False):
                # early blocks run right after the transition, while the
                # strict-FIFO DVE queue is still draining the last stripe's
                # rope; their mask/acc chain uses the (slower but empty)
                # Pool engine so the AV matmuls aren't head-of-line blocked
                eng = nc.gpsimd if early else nc.vector
                # two heads interleaved: consecutive score (and AV) matmuls
                # for h0/h0+1 share their kT (v) stationary, halving the
                # LDWEIGHTS traffic, and the softmax chains of the two heads
                # hide each other's latency
                hh = (h0, h0 + 1)
                q_t = [q_sb[h][b * (S // STRIPE) + qc] for h in hh]
                nk = (qc + 1) * (STRIPE // P)
                npairs = nk // 2
                depth = DEPTH_PAIRS
                acc2 = [accpool.tile([P, 2 * STRIPE], BF16, tag="acc",
                                     name=f"acc2_{i}") for i in range(2)]
                ps_av = [avpsum.tile([P, STRIPE], F32, tag="av",
                                     name=f"ps_av{i}") for i in range(2)]
                pexps = {}

                def diag_off(kj):
                    # first valid q column for key chunk kj (causal trim at
                    # 128-column granularity; bf16 has no min-moving penalty)
                    r = kj - qc * (STRIPE // P)
                    return r * P if r > 0 else 0

                def do_av(kj):
                    si = b * (S // STRIPE) + kj // (STRIPE // P)
                    off = diag_off(kj)
                    for i in range(2):  # same v stationary back-to-back
                        pex2 = pexps[(kj // 2, i)]
                        nc.tensor.matmul(ps_av[i][:, off:STRIPE],
                                         v_st[si][:, kj % (STRIPE // P), :],
                                         pex2[:, (kj % 2) * STRIPE + off:
                                              (kj % 2 + 1) * STRIPE],
                                         start=(kj == 0), stop=(kj == nk - 1))
                    if kj % 2 == 1:
                        del pexps[(kj // 2, 0)], pexps[(kj // 2, 1)]

                for p in range(npairs):
                    kj0 = 2 * p
                    # per head one [128,1024] psum tile; per key chunk the
                    # two heads' score matmuls run back-to-back (shared LDW)
                    ps2 = [spsum.tile([P, 2 * STRIPE], F32, tag="s",
                                      name=f"ps2_{i}") for i in range(2)]
                    offs = []
                    for half in range(2):
                        kj = kj0 + half
                        si = b * (S // STRIPE) + kj // (STRIPE // P)
                        off_k = (kj % (STRIPE // P)) * P
                        off = diag_off(kj)
                        offs.append(off)
                        for i in range(2):
                            nc.tensor.matmul(
                                ps2[i][:, half * STRIPE + off:
                                       (half + 1) * STRIPE],
                                kT_st[si][:, ds(off_k, P)],
                                q_t[i][:, off:STRIPE],
                                start=True, stop=True)
                    if p >= depth:
                        do_av(2 * (p - depth))
                        do_av(2 * (p - depth) + 1)
                    filler(1)
                    for i in range(2):
                        pex2 = probs_pool.tile([P, 2 * STRIPE], BF16,
                                               tag="pexp", name=f"pex2_{i}")
                        if offs == [0, 0]:
                            # one wide exp over both banks
                            nc.scalar.activation(
                                pex2[:], ps2[i][:],
                                mybir.ActivationFunctionType.Exp, scale=SCALE)
                        else:
                            for half in range(2):
                                off = offs[half]
                                lo = half * STRIPE + off
                                hi = (half + 1) * STRIPE
                                if off > 0:
                                    # trimmed region must read as prob 0 for
                                    # the denominator (Pool engine is idle)
                                    nc.gpsimd.memset(
                                        pex2[:, half * STRIPE:lo], 0.0)
                                nc.scalar.activation(
                                    pex2[:, lo:hi], ps2[i][:, lo:hi],
                                    mybir.ActivationFunctionType.Exp,
                                    scale=SCALE)
                        for half in range(2):
                            r = kj0 + half - qc * (STRIPE // P)
                            if r >= 0:  # diagonal: causal 0/1 mask
                                off = offs[half]
                                lo = half * STRIPE + off
                                hi = (half + 1) * STRIPE
                                eng.tensor_mul(
                                    pex2[:, lo:hi], pex2[:, lo:hi],
                                    cmask_sb[:, r, off:STRIPE])
                        pexps[(p, i)] = pex2
                        if p == 0:
                            eng.tensor_copy(acc2[i][:], pex2[:])
                        else:
                            eng.tensor_add(acc2[i][:], acc2[i][:],
                                           pex2[:])
                for p in range(max(0, npairs - depth), npairs):
                    do_av(2 * p)
                    do_av(2 * p + 1)
                for i in range(2):
                    acc1 = accpool.tile([P, STRIPE], BF16, tag="acc",
                                        name=f"acc1_{i}")
                    eng.tensor_add(acc1[:], acc2[i][:, 0:STRIPE],
                                   acc2[i][:, STRIPE:2 * STRIPE])
                    ps_cs = cspsum.tile([P, STRIPE], F32, tag="o",
                                        name=f"ps_cs{i}")
                    nc.tensor.matmul(ps_cs[:], ones_sb[:], acc1[:],
                                     start=True, stop=True)
                    rec = accpool.tile([P, STRIPE], F32, tag="acc",
                                       name=f"rec_{i}")
                    nc.vector.reciprocal_approx_fast(rec[:], ps_cs[:])
                    nc.vector.tensor_mul(attnT_bq[b][qc][:, hh[i], :],
                                         ps_av[i][:], rec[:])

            # note: routing the first blocks' mask/acc to Pool to dodge the
            # DVE queue was tried and measured slower — Pool's ~3x elemwise
            # cost exceeds the queue wait; keep everything on DVE
            n_blocks = 0
            for b in range(B):
                for qc in range(S // STRIPE):
                    for h0 in (0, 2):
                        attn_group2(b, h0, qc)
                        n_blocks += 1
                        if n_blocks == 2:
                            # transition crunch is over: emit the deferred
                            # stripe-7 q ropes (consumed by the last blocks)
                            for h, st7, qc_, cs_, sn_ in deferred_ropes:
                                rope(q_sb[h][st7][0:64, :],
                                     q_sb[h][st7][64:128, :],
                                     qc_[:], cs_[:], sn_[:])
                            deferred_ropes.clear()
                    # all 4 heads of (b, qc) done: its out tiles become ready
                    ready_tiles.extend(
                        ((b * (TOK_CHUNKS // 2) + qc * 4 + j), np2)
                        for j in range(4) for np2 in range(DIM // (2 * STRIPE)))
            while ready_tiles:
                out_tile(*ready_tiles.popleft())

    nc.compile()
    return nc


def _get_nc(loop_n: int = 1):
    key = ("nc", loop_n)
    if key not in _NC_CACHE:
        _NC_CACHE[key] = build_nc(loop_n)
    return _NC_CACHE[key]


def _host_prep(x, wq, wk, wv, wo, freqs_cos, freqs_sin):
    x = np.ascontiguousarray(np.asarray(x, dtype=np.float32))
    wq = np.asarray(wq, dtype=np.float32)
    wk = np.asarray(wk, dtype=np.float32)
    wv = np.asarray(wv, dtype=np.float32)
    wo = np.asarray(wo, dtype=np.float32)
    cos = np.asarray(freqs_cos, dtype=np.float32)
    sin = np.asarray(freqs_sin, dtype=np.float32)

    bf = ml_dtypes.bfloat16
    xT = np.ascontiguousarray(x.reshape(T, DIM).T).astype(bf)
    cosT = np.ascontiguousarray(np.concatenate([cos.T] * B, axis=1))
    sinT = np.ascontiguousarray(np.concatenate([sin.T] * B, axis=1))
    perm = np.concatenate([np.arange(0, HD, 2), np.arange(1, HD, 2)])
    km = np.arange(P)[:, None, None]
    rr = np.arange(4)[None, :, None]
    qn = np.arange(STRIPE)[None, None, :]
    cmask_np = ((rr * P + km) <= qn).astype(np.float32)

    in_maps = []
    for core in range(N_CORES):
        wq_i = wq[core * QD:(core + 1) * QD]
        wq_p = wq_i.reshape(N_HEADS_LOCAL, HD, DIM)[:, perm, :].reshape(QD, DIM)
        wk_p = wk[core * HD:(core + 1) * HD][perm, :]
        wv_i = wv[core * HD:(core + 1) * HD]
        wo_i = wo[:, core * QD:(core + 1) * QD]
        in_maps.append({
            "xT": xT,
            "wqT": np.ascontiguousarray(wq_p.T).astype(bf),
            "wkT": np.ascontiguousarray(wk_p.T).astype(bf),
            "wvT": np.ascontiguousarray(wv_i.T).astype(bf),
            "woT": np.ascontiguousarray(wo_i.T).astype(bf),
            "cosT": cosT,
            "sinT": sinT,
            "cmask": cmask_np.astype(bf),
        })
    return in_maps


def kernel(x, wq, wk, wv, wo, freqs_cos, freqs_sin, mask=None, start_pos=0):
    in_maps = _host_prep(x, wq, wk, wv, wo, freqs_cos, freqs_sin)
    nc = _get_nc()
    res = run_bass_kernel_spmd(nc, in_maps, list(range(N_CORES)))
    total = np.zeros((T, DIM), dtype=np.float64)
    for core in range(N_CORES):
        total += res.results[core]["out"]
    return total.astype(np.float32).reshape(B, S, DIM)


# revision 57
# speedup vs baseline: 1.0174x; 1.0174x over previous
"""Tensor-parallel GQA attention prefill for 8 TRN2 NeuronCores.

Shards the 32 Q heads / 8 KV heads across 8 cores (4 Q heads + 1 KV head
per core, kv-groups intact). Each core computes its heads' attention and
a partial output projection; the host sums the 8 partials.

Device-side layout choices (all prepared host-side):
 - x is passed transposed (xT [dim, tok]) so the contraction dim of the
   QKV projections lands on SBUF partitions with contiguous DMA.
 - wq/wk rows are permuted within each head to [even dims, odd dims] so
   RoPE's interleaved pairs become two contiguous 64-partition blocks.
   The permutation cancels in q.k dot products.
 - Projections produce qT/kT [d, tok]; scores are computed transposed
   (scoresT [ktok, qtok]) so softmax sums use a ones-matmul and the AV
   matmul needs no transposes. v is produced via PE-transpose of vT.
 - Causality is exploited structurally: upper-triangle score tiles are
   never computed; diagonal tiles are masked with a host-provided 0/1
   mask multiplied after exp (exp is overflow-safe at this scale, so no
   max subtraction is needed).
 - All matmul operands are bf16 (same 1 cycle/row PE rate as float32r,
   but FWL weight loads, half the DMA traffic and half the SBUF);
   PSUM accumulation and the softmax denominator stay fp32.
 - q stripes stay resident in SBUF between the projection and attention
   phases (32KB/partition in bf16) instead of a DRAM round trip.
 - PSUM->SBUF out-tile evictions alternate ACT/DVE (Pool cannot touch
   PSUM); the softmax accumulation runs in bf16 on DVE.
 - LDWEIGHTS exposure (~35ns per stationary change, measured) is cut by
   sharing stationaries: attention runs two heads interleaved (score/AV
   matmuls for both heads reuse each kT/v chunk), and the out projection
   emits two adjacent column slices per attnT stationary, h-major.
 - Diagonal score/AV matmuls skip causally-dead 128-column blocks; the
   trimmed prob regions are zeroed by Pool-engine memsets.
"""

import math
from contextlib import ExitStack

import ml_dtypes
import numpy as np

import concourse.bass as bass
import concourse.mybir as mybir
import concourse.tile as tile
from concourse import bacc
from concourse.bass import ts, ds
from concourse.bass_utils import run_bass_kernel_spmd
from concourse.masks import make_identity

P = 128
DIM = 4096
T = 4096          # b*s tokens, b-major
B = 2
S = 2048
N_HEADS_LOCAL = 4     # q heads per core
HD = 128              # head dim
QD = N_HEADS_LOCAL * HD   # 512 local q dim
N_CORES = 8
STRIPE = 512          # token stripe for projections / q chunks
N_STRIPES = T // STRIPE       # 8
K_CHUNKS = DIM // P           # 32
TOK_CHUNKS = T // P           # 32
SCALE = 1.0 / math.sqrt(HD)

F32 = mybir.dt.float32
F32R = mybir.dt.float32r
BF16 = mybir.dt.bfloat16

_NC_CACHE = {}


def build_nc(loop_n: int = 1):
    nc = bacc.Bacc("TRN2", target_bir_lowering=False, debug=False)

    xT = nc.dram_tensor("xT", [DIM, T], BF16, kind="ExternalInput").ap()
    wqT = nc.dram_tensor("wqT", [DIM, QD], BF16, kind="ExternalInput").ap()
    wkT = nc.dram_tensor("wkT", [DIM, HD], BF16, kind="ExternalInput").ap()
    wvT = nc.dram_tensor("wvT", [DIM, HD], BF16, kind="ExternalInput").ap()
    woT = nc.dram_tensor("woT", [QD, DIM], BF16, kind="ExternalInput").ap()
    cosT = nc.dram_tensor("cosT", [64, T], F32, kind="ExternalInput").ap()
    sinT = nc.dram_tensor("sinT", [64, T], F32, kind="ExternalInput").ap()
    cmask = nc.dram_tensor("cmask", [P, 4, STRIPE], BF16, kind="ExternalInput").ap()
    out = nc.dram_tensor("out", [T, DIM], F32, kind="ExternalOutput").ap()

    with tile.TileContext(nc) as tc, ExitStack() as octx:
        # ---- tensors that live across phases ----
        resident = octx.enter_context(tc.tile_pool(name="resident", bufs=1))
        # per-stripe k/v tiles: an attention group's reads depend on exactly
        # the stripes it consumes, not the whole phase-1 sweep
        kT_st = [resident.tile([P, STRIPE], BF16, tag=f"kT{st}", name=f"kT{st}")
                 for st in range(N_STRIPES)]                      # 8KB/part
        v_st = [resident.tile([P, STRIPE // P, HD], BF16, tag=f"v{st}",
                              name=f"v{st}")
                for st in range(N_STRIPES)]                       # 8KB/part
        # q stripes stay in SBUF: [head][stripe] -> [128, 512] bf16
        q_sb = [[resident.tile([P, STRIPE], BF16, tag=f"q{h}_{st}",
                               name=f"q{h}_{st}")
                 for st in range(N_STRIPES)] for h in range(N_HEADS_LOCAL)]
        ones_sb = resident.tile([P, P], BF16, tag="ones")
        ones_f32 = resident.tile([P, P], F32, tag="ones_f32")
        ident_sb = resident.tile([P, P], F32, tag="ident")
        cmask_sb = resident.tile([P, 4, STRIPE], BF16, tag="cmask")
        nc.gpsimd.memset(ones_f32[:], 1.0)
        nc.vector.tensor_copy(ones_sb[:], ones_f32[:])
        make_identity(nc, ident_sb[:])
        nc.sync.dma_start(cmask_sb[:], cmask)

        # phase-2/3 working pools live in the outer scope: allocating them
        # inside the phase would stall on the phase-1 pool boundary (all of
        # phase 1's SBUF consumers must drain before the space is reusable)
        probs_pool = octx.enter_context(tc.tile_pool(name="probs", bufs=8))
        accpool = octx.enter_context(tc.tile_pool(name="acc", bufs=6))
        opool = octx.enter_context(tc.tile_pool(name="outt", bufs=8))
        # wo lives in the outer scope (bf16 leaves SBUF room) so the first
        # out-proj fillers right after the transition don't wait on a 4MB
        # DMA; the load itself is issued mid-phase-1, off the startup path
        wopool = octx.enter_context(tc.tile_pool(name="weights3", bufs=1))
        wo_sb = wopool.tile([P, N_HEADS_LOCAL, DIM], BF16, tag="wo")  # 32KB
        # rope inputs live in the outer scope so the last stripe's q1-3
        # ropes can be deferred into phase 2 (their tiles must survive the
        # phase-1 pool boundary)
        cspool = octx.enter_context(tc.tile_pool(name="cossin", bufs=1))
        evict = octx.enter_context(tc.tile_pool(name="evict", bufs=4))
        rtmp = octx.enter_context(tc.tile_pool(name="rope_tmp", bufs=2))

        def rope(dst_hi, dst_lo, src, cos_s, sin_s):
            # src [128, STRIPE] SBUF fp32: rows 0:64 = t0 (even dims),
            # 64:128 = t1. cos_s/sin_s are [128, STRIPE] with the same 64
            # rows duplicated into both halves so every tensor_tensor's
            # two SBUF inputs share a base partition (walrus NCC_IBIR297).
            t0, t1 = src[0:64, :], src[64:128, :]
            a = rtmp.tile([64, STRIPE], F32, tag="rt", name="ra")
            b_ = rtmp.tile([64, STRIPE], F32, tag="rt", name="rb")
            nc.vector.tensor_mul(a[:], t0, cos_s[0:64, :])
            nc.vector.tensor_mul(b_[:], t1, sin_s[64:128, :])
            nc.vector.tensor_sub(dst_hi, a[:], b_[:])
            c_ = rtmp.tile([64, STRIPE], F32, tag="rt", name="rc")
            d_ = rtmp.tile([64, STRIPE], F32, tag="rt", name="rd")
            nc.vector.tensor_mul(c_[:], t0, sin_s[0:64, :])
            nc.vector.tensor_mul(d_[:], t1, cos_s[64:128, :])
            nc.vector.tensor_add(dst_lo, c_[:], d_[:])

        deferred_ropes = []   # (h, st, qcop, cos_s, sin_s) emitted in phase 2

        if loop_n > 1:   # timing builds: repeat the whole body on-device
            octx.enter_context(tc.For_i(0, loop_n, 1))

        # ================= phase 1: projections + rope =================
        with ExitStack() as ctx:
            wpool = ctx.enter_context(tc.tile_pool(name="weights1", bufs=1))
            xpool = ctx.enter_context(tc.tile_pool(name="xk", bufs=3))
            qpsum = ctx.enter_context(tc.tile_pool(name="q_psum", bufs=4, space="PSUM"))
            kpsum = ctx.enter_context(tc.tile_pool(name="k_psum", bufs=1, space="PSUM"))
            vpsum = ctx.enter_context(tc.tile_pool(name="v_psum", bufs=2, space="PSUM"))
            tpsum = ctx.enter_context(tc.tile_pool(name="tr_psum", bufs=1, space="PSUM"))
            vt_pool = ctx.enter_context(tc.tile_pool(name="vt", bufs=2))

            wq_sb = wpool.tile([P, K_CHUNKS, QD], BF16, tag="wq")   # 32KB/part
            wk_sb = wpool.tile([P, K_CHUNKS, HD], BF16, tag="wk")   # 8KB
            wv_sb = wpool.tile([P, K_CHUNKS, HD], BF16, tag="wv")   # 8KB

            def _proj_chunk(st, k, xk, psq, psk, psv, prev_vt):
                if st == 0 and k % 2 == 0:
                    # paired per-chunk weight loads (2 chunks per transfer):
                    # matmul k waits only on its own pair, and half the
                    # per-DMA fixed cost on the startup-critical path
                    nc.sync.dma_start(
                        wq_sb[:, k:k + 2, :],
                        wqT[ds(k * P, 2 * P), :].rearrange("(j p) c -> p j c", p=P))
                    nc.sync.dma_start(
                        wk_sb[:, k:k + 2, :],
                        wkT[ds(k * P, 2 * P), :].rearrange("(j p) c -> p j c", p=P))
                    nc.sync.dma_start(
                        wv_sb[:, k:k + 2, :],
                        wvT[ds(k * P, 2 * P), :].rearrange("(j p) c -> p j c", p=P))
                st_first, st_last = (k == 0), (k == K_CHUNKS - 1)
                for h in range(N_HEADS_LOCAL):
                    nc.tensor.matmul(psq[h][:], wq_sb[:, k, ts(h, HD)], xk,
                                     start=st_first, stop=st_last)
                nc.tensor.matmul(psk[:], wk_sb[:, k, :], xk,
                                 start=st_first, stop=st_last)
                nc.tensor.matmul(psv[:], wv_sb[:, k, :], xk,
                                 start=st_first, stop=st_last)
                # previous stripe's v transposes: deps met long ago, sit
                # between accumulation matmuls without stalling PE
                if k == 0 and prev_vt is not None:
                    pvt, pvt_st = prev_vt
                    for j in range(STRIPE // P):
                        pstt = tpsum.tile([P, P], F32, tag="pst",
                                          name=f"pst{j}")
                        nc.tensor.transpose(pstt[:], pvt[:, ts(j, P)],
                                            ident_sb[:])
                        nc.scalar.copy(v_st[pvt_st][:, j, :], pstt[:])

            prev_vt = None
            for st in range(N_STRIPES):
                tok = ts(st, STRIPE)
                psq = [qpsum.tile([P, STRIPE], F32, tag="psq", name=f"psq{i}")
                       for i in range(N_HEADS_LOCAL)]
                psk = kpsum.tile([P, STRIPE], F32, tag="psk")
                psv = vpsum.tile([P, STRIPE], F32, tag="psv")
                for k2 in range(K_CHUNKS // 2):
                    # two k-chunks per DMA: halves the per-transfer fixed
                    # costs on the phase-1 feed path
                    xk2 = xpool.tile([P, 2, STRIPE], BF16, tag="xk")
                    nc.sync.dma_start(
                        xk2[:], xT[ds(k2 * 2 * P, 2 * P), tok].rearrange(
                            "(j p) t -> p j t", p=P))
                    for j in range(2):
                        k = 2 * k2 + j
                        _proj_chunk(st, k, xk2[:, j, :], psq, psk, psv,
                                    prev_vt)

                # evict PSUM -> SBUF fast on the Pool engine so next stripe's
                # matmuls get their PSUM banks back quickly
                kcop = evict.tile([P, STRIPE], F32, tag="kcop")
                nc.scalar.copy(kcop[:], psk[:])
                vt = vt_pool.tile([P, STRIPE], F32, tag="vt")
                nc.scalar.copy(vt[:], psv[:])
                qcop = []
                for h in range(N_HEADS_LOCAL):
                    qc_ = evict.tile([P, STRIPE], F32, tag="kcop",
                                     name=f"qcop{h}")
                    nc.scalar.copy(qc_[:], psq[h][:])
                    qcop.append(qc_)

                cos_s = cspool.tile([P, STRIPE], F32, tag="cos")
                sin_s = cspool.tile([P, STRIPE], F32, tag="sin")
                nc.sync.dma_start(cos_s[0:64, :], cosT[:, tok])
                nc.sync.dma_start(sin_s[0:64, :], sinT[:, tok])
                nc.scalar.copy(cos_s[64:128, :], cos_s[0:64, :])
                nc.scalar.copy(sin_s[64:128, :], sin_s[0:64, :])

                if st == 2:   # past the startup DMA crunch, long before use
                    for hh in range(N_HEADS_LOCAL):
                        nc.sync.dma_start(wo_sb[:, hh, :], woT[ts(hh, P), :])

                rope(kT_st[st][0:64, :], kT_st[st][64:128, :], kcop[:],
                     cos_s[:], sin_s[:])
                for h in range(N_HEADS_LOCAL):
                    if st == N_STRIPES - 1 and h >= 1:
                        # stripe 7's q1-3 feed only the very last attention
                        # blocks; deferring their ropes into phase 2 keeps
                        # ~18 TensorTensor ops out of the strict-FIFO DVE
                        # queue at the phase transition, so the first
                        # blocks' mask/acc (and thus AV matmuls) start
                        # immediately
                        deferred_ropes.append((h, st, qcop[h], cos_s, sin_s))
                    else:
                        rope(q_sb[h][st][0:64, :], q_sb[h][st][64:128, :],
                             qcop[h][:], cos_s[:], sin_s[:])
                prev_vt = (vt, st)

            # last stripe's v transposes
            pvt, pvt_st = prev_vt
            for j in range(STRIPE // P):
                pstt = tpsum.tile([P, P], F32, tag="pst", name=f"pstz{j}")
                nc.tensor.transpose(pstt[:], pvt[:, ts(j, P)], ident_sb[:])
                nc.scalar.copy(v_st[pvt_st][:, j, :], pstt[:])

        # ================= phase 2+3: attention + out proj =================
        with ExitStack() as ctx:
            atpool = ctx.enter_context(tc.tile_pool(name="attnT", bufs=1))
            spsum = ctx.enter_context(tc.tile_pool(name="s_psum", bufs=2, space="PSUM"))
            avpsum = ctx.enter_context(tc.tile_pool(name="av_psum", bufs=2, space="PSUM"))
            opsum = ctx.enter_context(tc.tile_pool(name="o_psum", bufs=2, space="PSUM"))
            cspsum = opsum  # colsum tiles share the out-proj psum slots

            # per-(b,qc) tiles so an out tile depends on exactly its 4 head
            # writes, not every norm traced before it
            attnT_bq = [[atpool.tile([P, N_HEADS_LOCAL, STRIPE], BF16,
                                     tag=f"attnT{b}_{qc}", name=f"attnT{b}_{qc}")
                         for qc in range(S // STRIPE)] for b in range(B)]  # 32KB

            DEPTH_PAIRS = 3   # score2 -> exp2 -> av pipeline depth (in kj pairs)
            from collections import deque
            ready_tiles = deque()

            def out_tile(tc32, np2, pool_=None, ptag="o"):
                # two adjacent out-column slices per call, matmuls h-major:
                # consecutive matmuls share their attnT stationary operand,
                # halving the LDWEIGHTS traffic of the output projection
                if pool_ is None:
                    pool_ = opsum
                b, qc = tc32 // (TOK_CHUNKS // 2), (tc32 % (TOK_CHUNKS // 2)) // 4
                ps_oA = pool_.tile([P, STRIPE], F32, tag=ptag, name="ps_oA")
                ps_oB = pool_.tile([P, STRIPE], F32, tag=ptag, name="ps_oB")
                for h in range(N_HEADS_LOCAL):
                    st_f, st_l = (h == 0), (h == N_HEADS_LOCAL - 1)
                    stat = attnT_bq[b][qc][:, h, ts(tc32 % 4, P)]
                    nc.tensor.matmul(ps_oA[:], stat,
                                     wo_sb[:, h, ts(2 * np2, STRIPE)],
                                     start=st_f, stop=st_l)
                    nc.tensor.matmul(ps_oB[:], stat,
                                     wo_sb[:, h, ts(2 * np2 + 1, STRIPE)],
                                     start=st_f, stop=st_l)
                o_sbA = opool.tile([P, STRIPE], F32, tag="o_sb", name="o_sbA")
                o_sbB = opool.tile([P, STRIPE], F32, tag="o_sb", name="o_sbB")
                # parallel PSUM evictions: ACT takes one, DVE the other
                nc.scalar.copy(o_sbA[:], ps_oA[:])
                nc.vector.tensor_copy(o_sbB[:], ps_oB[:])
                nc.sync.dma_start(out[ts(tc32, P), ts(2 * np2, STRIPE)],
                                  o_sbA[:])
                nc.sync.dma_start(out[ts(tc32, P), ts(2 * np2 + 1, STRIPE)],
                                  o_sbB[:])

            def filler(nmax):
                for _ in range(min(nmax, len(ready_tiles))):
                    out_tile(*ready_tiles.popleft())

            def attn_group2(b, h0, qc, early=False):
                # early blocks run right after the transition, while the
                # strict-FIFO DVE queue is still draining the last stripe's
                # rope; their mask/acc chain uses the (slower but empty)
                # Pool engine so the AV matmuls aren't head-of-line blocked
                eng = nc.gpsimd if early else nc.vector
                # two heads interleaved: consecutive score (and AV) matmuls
                # for h0/h0+1 share their kT (v) stationary, halving the
                # LDWEIGHTS traffic, and the softmax chains of the two heads
                # hide each other's latency
                hh = (h0, h0 + 1)
                q_t = [q_sb[h][b * (S // STRIPE) + qc] for h in hh]
                nk = (qc + 1) * (STRIPE // P)
                npairs = nk // 2
                depth = DEPTH_PAIRS
                acc2 = [accpool.tile([P, 2 * STRIPE], BF16, tag="acc",
                                     name=f"acc2_{i}") for i in range(2)]
                ps_av = [avpsum.tile([P, STRIPE], F32, tag="av",
                                     name=f"ps_av{i}") for i in range(2)]
                pexps = {}

                def diag_off(kj):
                    # first valid q column for key chunk kj (causal trim at
                    # 128-column granularity; bf16 has no min-moving penalty)
                    r = kj - qc * (STRIPE // P)
                    return r * P if r > 0 else 0

                def do_av(kj):
                    si = b * (S // STRIPE) + kj // (STRIPE // P)
                    off = diag_off(kj)
                    for i in range(2):  # same v stationary back-to-back
                        pex2 = pexps[(kj // 2, i)]
                        nc.tensor.matmul(ps_av[i][:, off:STRIPE],
                                         v_st[si][:, kj % (STRIPE // P), :],
                                         pex2[:, (kj % 2) * STRIPE + off:
                                              (kj % 2 + 1) * STRIPE],
                                         start=(kj == 0), stop=(kj == nk - 1))
                    if kj % 2 == 1:
                        del pexps[(kj // 2, 0)], pexps[(kj // 2, 1)]

                for p in range(npairs):
                    kj0 = 2 * p
                    # per head one [128,1024] psum tile; per key chunk the
                    # two heads' score matmuls run back-to-back (shared LDW)
                    ps2 = [spsum.tile([P, 2 * STRIPE], F32, tag="s",
                                      name=f"ps2_{i}") for i in range(2)]
                    offs = []
                    for half in range(2):
                        kj = kj0 + half
                        si = b * (S // STRIPE) + kj // (STRIPE // P)
                        off_k = (kj % (STRIPE // P)) * P
                        off = diag_off(kj)
                        offs.append(off)
                        for i in range(2):
                            nc.tensor.matmul(
                                ps2[i][:, half * STRIPE + off:
                                       (half + 1) * STRIPE],
                                kT_st[si][:, ds(off_k, P)],
                                q_t[i][:, off:STRIPE],
                                start=True, stop=True)
                    if p >= depth:
                        do_av(2 * (p - depth))
                        do_av(2 * (p - depth) + 1)
                    filler(1)
                    for i in range(2):
                        pex2 = probs_pool.tile([P, 2 * STRIPE], BF16,
                                               tag="pexp", name=f"pex2_{i}")
                        if offs == [0, 0]:
                            # one wide exp over both banks
                            nc.scalar.activation(
                                pex2[:], ps2[i][:],
                                mybir.ActivationFunctionType.Exp, scale=SCALE)
                        else:
                            for half in range(2):
                                off = offs[half]
                                lo = half * STRIPE + off
                                hi = (half + 1) * STRIPE
                                if off > 0:
                                    # trimmed region must read as prob 0 for
                                    # the denominator (Pool engine is idle)
                                    nc.gpsimd.memset(
                                        pex2[:, half * STRIPE:lo], 0.0)
                                nc.scalar.activation(
                                    pex2[:, lo:hi], ps2[i][:, lo:hi],
                                    mybir.ActivationFunctionType.Exp,
                                    scale=SCALE)
                        for half in range(2):
                            r = kj0 + half - qc * (STRIPE // P)
                            if r >= 0:  # diagonal: causal 0/1 mask
                                off = offs[half]
                                lo = half * STRIPE + off
                                hi = (half + 1) * STRIPE
                                eng.tensor_mul(
                                    pex2[:, lo:hi], pex2[:, lo:hi],
                                    cmask_sb[:, r, off:STRIPE])
                        pexps[(p, i)] = pex2
                        if p == 0:
                            eng.tensor_copy(acc2[i][:], pex2[:])
                        else:
                            eng.tensor_add(acc2[i][:], acc2[i][:],
                                           pex2[:])
                for p in range(max(0, npairs - depth), npairs):
                    do_av(2 * p)
                    do_av(2 * p + 1)
                for i in range(2):
                    acc1 = accpool.tile([P, STRIPE], BF16, tag="acc",
                                        name=f"acc1_{i}")
                    eng.tensor_add(acc1[:], acc2[i][:, 0:STRIPE],
                                   acc2[i][:, STRIPE:2 * STRIPE])
                    ps_cs = cspsum.tile([P, STRIPE], F32, tag="o",
                                        name=f"ps_cs{i}")
                    nc.tensor.matmul(ps_cs[:], ones_sb[:], acc1[:],
                                     start=True, stop=True)
                    rec = accpool.tile([P, STRIPE], F32, tag="acc",
                                       name=f"rec_{i}")
                    nc.vector.reciprocal_approx_fast(rec[:], ps_cs[:])
                    nc.vector.tensor_mul(attnT_bq[b][qc][:, hh[i], :],
                                         ps_av[i][:], rec[:])

            # note: routing the first blocks' mask/acc to Pool to dodge the
            # DVE queue was tried and measured slower — Pool's ~3x elemwise
            # cost exceeds the queue wait; keep everything on DVE
            n_blocks = 0
            for b in range(B):
                for qc in range(S // STRIPE):
                    for h0 in (0, 2):
                        attn_group2(b, h0, qc)
                        n_blocks += 1
                        if n_blocks == 2:
                            # transition crunch is over: emit the deferred
                            # stripe-7 q ropes (consumed by the last blocks)
                            for h, st7, qc_, cs_, sn_ in deferred_ropes:
                                rope(q_sb[h][st7][0:64, :],
                                     q_sb[h][st7][64:128, :],
                                     qc_[:], cs_[:], sn_[:])
                            deferred_ropes.clear()
                    # all 4 heads of (b, qc) done: its out tiles become ready
                    ready_tiles.extend(
                        ((b * (TOK_CHUNKS // 2) + qc * 4 + j), np2)
                        for j in range(4) for np2 in range(DIM // (2 * STRIPE)))
            # final drain: attention is done, so the av psum ring is free —
            # alternate units between the o and av rings so each unit's
            # evictions overlap the next unit's matmuls instead of gating
            # them (4 banks in rotation instead of 2)
            drain_i = 0
            while ready_tiles:
                tc32, np2 = ready_tiles.popleft()
                if drain_i % 2 == 0:
                    out_tile(tc32, np2)
                else:
                    out_tile(tc32, np2, pool_=avpsum, ptag="av")
                drain_i += 1

    nc.compile()
    return nc


def _get_nc(loop_n: int = 1):
    key = ("nc", loop_n)
    if key not in _NC_CACHE:
        _NC_CACHE[key] = build_nc(loop_n)
    return _NC_CACHE[key]


def _host_prep(x, wq, wk, wv, wo, freqs_cos, freqs_sin):
    x = np.ascontiguousarray(np.asarray(x, dtype=np.float32))
    wq = np.asarray(wq, dtype=np.float32)
    wk = np.asarray(wk, dtype=np.float32)
    wv = np.asarray(wv, dtype=np.float32)
    wo = np.asarray(wo, dtype=np.float32)
    cos = np.asarray(freqs_cos, dtype=np.float32)
    sin = np.asarray(freqs_sin, dtype=np.float32)

    bf = ml_dtypes.bfloat16
    xT = np.ascontiguousarray(x.reshape(T, DIM).T).astype(bf)
    cosT = np.ascontiguousarray(np.concatenate([cos.T] * B, axis=1))
    sinT = np.ascontiguousarray(np.concatenate([sin.T] * B, axis=1))
    perm = np.concatenate([np.arange(0, HD, 2), np.arange(1, HD, 2)])
    km = np.arange(P)[:, None, None]
    rr = np.arange(4)[None, :, None]
    qn = np.arange(STRIPE)[None, None, :]
    cmask_np = ((rr * P + km) <= qn).astype(np.float32)

    in_maps = []
    for core in range(N_CORES):
        wq_i = wq[core * QD:(core + 1) * QD]
        wq_p = wq_i.reshape(N_HEADS_LOCAL, HD, DIM)[:, perm, :].reshape(QD, DIM)
        wk_p = wk[core * HD:(core + 1) * HD][perm, :]
        wv_i = wv[core * HD:(core + 1) * HD]
        wo_i = wo[:, core * QD:(core + 1) * QD]
        in_maps.append({
            "xT": xT,
            "wqT": np.ascontiguousarray(wq_p.T).astype(bf),
            "wkT": np.ascontiguousarray(wk_p.T).astype(bf),
            "wvT": np.ascontiguousarray(wv_i.T).astype(bf),
            "woT": np.ascontiguousarray(wo_i.T).astype(bf),
            "cosT": cosT,
            "sinT": sinT,
            "cmask": cmask_np.astype(bf),
        })
    return in_maps


def kernel(x, wq, wk, wv, wo, freqs_cos, freqs_sin, mask=None, start_pos=0):
    in_maps = _host_prep(x, wq, wk, wv, wo, freqs_cos, freqs_sin)
    nc = _get_nc()
    res = run_bass_kernel_spmd(nc, in_maps, list(range(N_CORES)))
    total = np.zeros((T, DIM), dtype=np.float64)
    for core in range(N_CORES):
        total += res.results[core]["out"]
    return total.astype(np.float32).reshape(B, S, DIM)
